# revision 3
# baseline (speedup 1.0000x reference)
"""Trainium2 Bass kernel for nn_BSplineActivation.

y(x) = sum_j B_j(x) w_j for a degree-3 B-spline on a uniform knot grid
(1024 knots on [-pi, pi], fp32). Per point only 4 basis functions are
non-zero, so y restricted to knot interval i is a cubic polynomial.

Strategy:
  * Host (weights-only preprocessing): build a [1023, 8] table with, per
    interval i, the exact cubic coefficients (c0..c3) of y expanded
    around m_i = knots32[i], plus m_i itself. Built in float64 from the
    float32 knot values, so the device result matches the reference up
    to fp32 rounding.
  * Device (all per-point work): z = x*inv_h + C, clamp, floor -> i;
    gather table row i (indirect DMA, one row per point); u = x - m_i;
    Horner; mask x outside [knot0, knot_last).
  * Data parallel over 8 NeuronCores: x is split into 8 shards of 32768
    points; the table is replicated.
"""
import sys

sys.path.insert(0, "/opt/trn_rl_repo")

import numpy as np

import concourse.bacc as bacc
import concourse.mybir as mybir
import concourse.tile as tile
from concourse.bass import IndirectOffsetOnAxis
from concourse.bass_utils import run_bass_kernel_spmd

P, F = 128, 256          # per-core layout: 128 partitions x 256 points
NCORES = 8
NPTS = NCORES * P * F    # 262144
NUM_KNOTS = 1024
DEGREE = 3
NW = NUM_KNOTS - DEGREE - 1  # 1020 weights
NI = NUM_KNOTS - 1           # 1023 intervals

f32 = mybir.dt.float32
i32 = mybir.dt.int32
AL = mybir.AluOpType

_KNOTS32 = np.linspace(-np.pi, np.pi, NUM_KNOTS).astype(np.float32)
_T0 = float(_KNOTS32[0])
_TLAST = float(_KNOTS32[-1])
# z = x * INV_H + CB maps x to the (approximate) interval coordinate.
_H64 = (float(_KNOTS32[-1]) - float(_KNOTS32[0])) / float(NI)
_INV_H = float(np.float32(1.0 / _H64))
_CB = float(np.float32(-float(_KNOTS32[0]) / _H64))


def _bspline_basis_f64(x, knots, degree):
    """Reference Cox-de Boor recursion in float64 (on fp32 knot values)."""
    t = knots.astype(np.float64)
    n = t.shape[0] - 1
    xe = x[:, None]
    B = ((t[:-1] <= xe) & (xe < t[1:])).astype(np.float64)
    for k in range(1, degree + 1):
        d1 = t[k:n] - t[: n - k]
        d2 = t[k + 1 : n + 1] - t[1 : n - k + 1]
        w1 = np.where(d1 > 0, (xe - t[: n - k]) / np.where(d1 > 0, d1, 1.0), 0.0)
        w2 = np.where(d2 > 0, (t[k + 1 : n + 1] - xe) / np.where(d2 > 0, d2, 1.0), 0.0)
        B = w1 * B[:, : n - k] + w2 * B[:, 1 : n - k + 1]
    return B


def _build_table(weights: np.ndarray) -> np.ndarray:
    """[1023, 8] fp32: per interval i: c0,c1,c2,c3 (in u = x - m_i), m_i, pad."""
    w64 = weights.astype(np.float64)
    t64 = _KNOTS32.astype(np.float64)
    # 4 sample offsets inside each interval (fractions of the local width)
    fr = np.array([0.0625, 0.3125, 0.6875, 0.9375])
    lo = t64[:-1]
    wid = t64[1:] - t64[:-1]
    xs = lo[:, None] + wid[:, None] * fr[None, :]          # [1023, 4]
    ys = _bspline_basis_f64(xs.ravel(), _KNOTS32, DEGREE) @ w64
    ys = ys.reshape(NI, 4)
    # Fit exact cubic through the 4 samples, in normalized s = (x-m)/wid.
    V = np.stack([fr**k for k in range(4)], axis=-1)       # [4, 4]
    a = np.linalg.solve(V[None, :, :].repeat(NI, 0), ys[:, :, None])[:, :, 0]
    tab = np.zeros((NI, 8), dtype=np.float64)
    for k in range(4):
        tab[:, k] = a[:, k] / wid**k                       # back to u units
    tab[:, 4] = lo                                         # m_i = knots32[i]
    return tab.astype(np.float32)


_NC_CACHE = {}


def _build_nc(reps: int = 1):
    """reps > 1 repeats the full compute body (for device-time estimation)."""
    nc = bacc.Bacc("TRN2", target_bir_lowering=False, debug=False, num_devices=NCORES)
    x_d = nc.dram_tensor("x", [P, F], f32, kind="ExternalInput")
    tab_d = nc.dram_tensor("tab", [NI, 8], f32, kind="ExternalInput")
    y_d = nc.dram_tensor("y", [P, F], f32, kind="ExternalOutput")
    with tile.TileContext(nc) as tc:
        with tc.tile_pool(name="sbuf", bufs=1) as pool:
            xt = pool.tile([P, F], f32)
            nc.sync.dma_start(xt[:], x_d.ap()[:])
            cb = pool.tile([P, 1], f32)
            nc.vector.memset(cb[:], _CB)
            yt = None
            for _rep in range(reps):
                # interval coordinate, clamped; floor via round(z - 0.5)
                zc = pool.tile([P, F], f32)
                nc.scalar.activation(zc[:], xt[:], mybir.ActivationFunctionType.Identity,
                                     bias=cb[:], scale=_INV_H)
                zf = pool.tile([P, F], f32)
                nc.vector.tensor_scalar(out=zf[:], in0=zc[:], scalar1=0.0, scalar2=1022.9995,
                                        op0=AL.max, op1=AL.min)
                idxt = pool.tile([P, F], i32)
                nc.vector.tensor_scalar(out=idxt[:], in0=zf[:], scalar1=0.5, scalar2=None,
                                        op0=AL.subtract)
                # gather rows: one [P,1]-offset indirect DMA per point column
                gath = pool.tile([P, F * 8], f32)
                for j in range(F):
                    nc.gpsimd.indirect_dma_start(
                        out=gath[:, 8 * j : 8 * j + 8],
                        out_offset=None,
                        in_=tab_d.ap()[:],
                        in_offset=IndirectOffsetOnAxis(ap=idxt[:, j : j + 1], axis=0),
                    )
                gv = gath[:].rearrange("p (f e) -> p f e", e=8)
                u = pool.tile([P, F], f32)
                nc.vector.tensor_tensor(out=u[:], in0=xt[:], in1=gv[:, :, 4], op=AL.subtract)
                acc = pool.tile([P, F], f32)
                nc.vector.tensor_tensor(out=acc[:], in0=gv[:, :, 3], in1=u[:], op=AL.mult)
                nc.vector.tensor_tensor(out=acc[:], in0=acc[:], in1=gv[:, :, 2], op=AL.add)
                nc.vector.tensor_tensor(out=acc[:], in0=acc[:], in1=u[:], op=AL.mult)
                nc.vector.tensor_tensor(out=acc[:], in0=acc[:], in1=gv[:, :, 1], op=AL.add)
                nc.vector.tensor_tensor(out=acc[:], in0=acc[:], in1=u[:], op=AL.mult)
                nc.vector.tensor_tensor(out=acc[:], in0=acc[:], in1=gv[:, :, 0], op=AL.add)
                # mask to zero outside [T0, TLAST)
                m1 = pool.tile([P, F], f32)
                nc.vector.scalar_tensor_tensor(out=m1[:], in0=xt[:], scalar=_TLAST,
                                               in1=acc[:], op0=AL.is_lt, op1=AL.mult)
                yt = pool.tile([P, F], f32)
                nc.vector.scalar_tensor_tensor(out=yt[:], in0=xt[:], scalar=_T0,
                                               in1=m1[:], op0=AL.is_ge, op1=AL.mult)
            nc.sync.dma_start(y_d.ap()[:], yt[:])
    nc.compile()
    return nc


def _in_maps(x, weights):
    tab = _build_table(np.asarray(weights))
    xs = np.ascontiguousarray(np.asarray(x, dtype=np.float32).reshape(NCORES, P, F))
    return [{"x": xs[c], "tab": tab} for c in range(NCORES)]


def kernel(x: np.ndarray, weights: np.ndarray) -> np.ndarray:
    if "nc" not in _NC_CACHE:
        _NC_CACHE["nc"] = _build_nc()
    nc = _NC_CACHE["nc"]
    res = run_bass_kernel_spmd(nc, _in_maps(x, weights), core_ids=list(range(NCORES)))
    y = np.stack([res.results[c]["y"] for c in range(NCORES)], axis=0)
    return y.reshape(NPTS, 1).astype(np.float32)


def estimate_hw_ns(x=None, weights=None, reps_hi: int = 3, timing_reps: int = 12) -> int:
    """Device time per kernel body: wall-clock delta between reps=1 and
    reps=reps_hi builds (amplification cancels host/launch overhead)."""
    import time as _time

    if x is None:
        rng = np.random.default_rng(0)
        x = rng.standard_normal((NPTS, 1)).astype(np.float32)
        weights = rng.standard_normal((1020,)).astype(np.float32)
    im = _in_maps(x, weights)
    walls = {}
    for reps in (1, reps_hi):
        nc = _NC_CACHE.get(("nc", reps))
        if nc is None:
            nc = _build_nc(reps) if reps > 1 else _NC_CACHE.get("nc") or _build_nc()
            _NC_CACHE[("nc", reps)] = nc
        run_bass_kernel_spmd(nc, im, core_ids=list(range(NCORES)))
        ts = []
        for _ in range(timing_reps):
            t0 = _time.perf_counter()
            run_bass_kernel_spmd(nc, im, core_ids=list(range(NCORES)))
            ts.append(_time.perf_counter() - t0)
        walls[reps] = min(ts)
    return int((walls[reps_hi] - walls[1]) / (reps_hi - 1) * 1e9)
